# revision 3
# baseline (speedup 1.0000x reference)
"""Trainium2 Bass kernel for nn_CoevolutionAnalyzer (pairwise-MLP coevolution scores).

Math (per batch q):
    g = domain * evo                         [512, 128]
    a = g @ W1[:128], c = g @ W1[128:]       [512, 128]
    h_ij  = relu(a_i + c_j + b1)             [128]
    z2    = W2.T h_ij + b2 ; h2 = relu(z2)   [64]
    s_ij  = sigmoid(W3.h2 + b3)
    out   = triu(s,1) + triu(s,1).T

Sharding (8 cores, one SPMD program):
    Only j >= 64*floor(i/64) is computed (upper triangle padded to the 64-column
    block grid; the pad is discarded on the host via triu). Every core takes 8
    rows of every 64-row block (rows 64*b + 8*k + m) for both batches, so all
    cores run the identical instruction stream; only DMA'd data differs.

Per block (8 rows = 4 pairs, j-window [64b, 512) of length L):
    stage1 (DVE/GpSimd): h = relu(cT + a_i + b1) as bf16, two stacked tiles
            (even rows / odd rows), G pairs side by side (G*L <= 512)
    stage2 (PE bf16):   two accumulating 128-wide matmuls with stationaries
            [W2|0] and [0|W2] -> psz[0:64]=z2_even, psz[64:128]=z2_odd.
            128-wide stationary => Fast Weight Load => LDWEIGHTS hidden.
    relu2  (ACT or DVE): h2 = relu(psum + [b2;b2]) -> bf16
    stage3 (PE bf16):   per round r one 128-wide matmul with stationary w3q_r
            (W3-even at col 32r, W3-odd at col 32r+1, zeros elsewhere),
            accumulated into pss over the R rounds of the block.
    sigmoid(ACT):       bf16 into a per-batch staging tile [128, 3328]
    out:                partitions 32r+{0,1} shipped per batch, trimmed per r
"""

import os

import numpy as np
from ml_dtypes import bfloat16 as bf16_np

import concourse.bass as bass
import concourse.tile as tile
from concourse import bacc, mybir
from concourse.bass_utils import run_bass_kernel_spmd

B = 2
N = 512
D = 128
NB = 8          # number of 64-row j-blocks
BLK = N // NB   # 64
RPB = 8         # rows per core per block
NI = NB * RPB   # i-rows per core per batch (64)
F32 = mybir.dt.float32
BF16 = mybir.dt.bfloat16
AF = mybir.ActivationFunctionType
ALU = mybir.AluOpType

# per-block geometry
LS = [N - BLK * b for b in range(NB)]            # j-window lengths
GS = [min(4, N // L) for L in LS]                # pairs per round
RS = [4 // g for g in GS]                        # rounds per block
WS = [GS[b] * LS[b] for b in range(NB)]          # sig segment widths
SIGW = sum(WS)                                   # 3328
SIGBASE = [sum(WS[:b]) for b in range(NB)]

# ---- engine balance knobs ----
GPS_ROWS = (6, 7)   # local rows m (0..7) whose stage-1 goes to GpSimd


def RELU2_ON_DVE(q, b):
    return (q == 1 and b in (0, 1)) or (q == 0 and b == 0)


# din column layout (host-reordered for early critical path)
DJ0, EJ0, DI, EI, DJ1, EJ1 = 0, N, 2 * N, 2 * N + NI * B, 2 * N + 2 * NI * B, 3 * N + 2 * NI * B
DIN_COLS = 4 * N + 2 * NI * B  # 2304

# how much of each a-strip is valid after block 6 (r < R_b blocks only)
SHIP1 = [SIGBASE[7], SIGBASE[6], SIGBASE[4], SIGBASE[4]]

LAST_RESULT = None  # set by kernel(); test harness reads exec_time_ns


def _build():
    nc = bacc.Bacc("TRN2", target_bir_lowering=False, debug=False, num_devices=8)

    din = nc.declare_dram_parameter("din", [D, DIN_COLS], BF16, isOutput=False)
    wf = nc.declare_dram_parameter("wf", [D, 2 * D], BF16, isOutput=False)    # w1a|w1b
    bb = nc.declare_dram_parameter("bb", [D, 3], F32, isOutput=False)         # b1|b2s|b3
    # wb: w2z0|w2z1|w3q0|w3q1|w3q2|w3q3  (each 128 cols)
    wb = nc.declare_dram_parameter("wb", [D, 6 * D], BF16, isOutput=False)
    out = nc.declare_dram_parameter("out", [B, 4, 2, SIGW], BF16, isOutput=True)

    with tile.TileContext(nc) as tc:
        with (
            tc.tile_pool(name="singles", bufs=1) as singles,
            tc.tile_pool(name="per_batch", bufs=2) as per_batch,
            tc.tile_pool(name="hpool", bufs=6) as hpool,
            tc.tile_pool(name="h2pool", bufs=3) as h2pool,
            tc.tile_pool(name="psz", bufs=2, space="PSUM") as psz_pool,
            tc.tile_pool(name="pss", bufs=2, space="PSUM") as pss_pool,
            tc.tile_pool(name="pset", bufs=1, space="PSUM") as pset_pool,
        ):
            s_in = singles.tile([D, DIN_COLS], BF16)
            s_wf = singles.tile([D, 2 * D], BF16)
            s_bb = singles.tile([D, 3], F32)
            s_wb = singles.tile([D, 6 * D], BF16)
            s_scr = singles.tile([D, 2], F32)

            # dummy sigmoid first so the act-table pass loads the
            # sigmoid_and_others set once (it also contains relu)
            nc.vector.memset(s_scr[:, 0:1], 0.0)
            nc.scalar.activation(
                out=s_scr[:, 1:2], in_=s_scr[:, 0:1], func=AF.Sigmoid
            )

            # critical-path input DMAs: batch-0 j-side first (SP queue),
            # weights/biases on ACT queue behind the table load
            nc.sync.dma_start(out=s_in[:, 0 : 2 * N], in_=din[:, 0 : 2 * N])
            nc.sync.dma_start(out=s_wf, in_=wf[:])
            nc.sync.dma_start(
                out=s_in[:, DI : DI + 2 * NI * B], in_=din[:, DI : DI + 2 * NI * B]
            )
            nc.scalar.dma_start(out=s_bb, in_=bb[:])
            nc.scalar.dma_start(out=s_wb, in_=wb[:])
            nc.scalar.dma_start(out=s_in[:, DJ1:], in_=din[:, DJ1:])

            s_w2z0 = s_wb[:, 0:D]
            s_w2z1 = s_wb[:, D : 2 * D]
            s_b1 = s_bb[:, 0:1]
            s_b2s = s_bb[:, 1:2]
            s_b3 = s_bb[:, 2:3]

            for q in range(B):
                # --- per-batch setup: gT, aT(+b1), cT ---
                gti = per_batch.tile([D, NI], BF16, tag="gti")
                nc.vector.tensor_mul(
                    gti,
                    s_in[:, DI + q * NI : DI + (q + 1) * NI],
                    s_in[:, EI + q * NI : EI + (q + 1) * NI],
                )
                dj = DJ0 if q == 0 else DJ1
                ej = EJ0 if q == 0 else EJ1
                gtj = per_batch.tile([D, N], BF16, tag="gtj")
                nc.vector.tensor_mul(
                    gtj, s_in[:, dj : dj + N], s_in[:, ej : ej + N]
                )
                ps_c = pset_pool.tile([D, N], F32, tag="ps_c")
                nc.tensor.matmul(ps_c[:], s_wf[:, D:], gtj[:])
                ps_a = pset_pool.tile([D, NI], F32, tag="ps_a")
                nc.tensor.matmul(ps_a[:], s_wf[:, :D], gti[:])
                ct = per_batch.tile([D, N], BF16, tag="ct")
                nc.scalar.copy(ct, ps_c[:])
                abt = per_batch.tile([D, NI], F32, tag="abt")
                nc.vector.tensor_scalar_add(abt, ps_a[:], s_b1)

                sig = per_batch.tile([D, SIGW], BF16, tag="sig")

                # --- j-block loop ---
                for b in range(NB):
                    j0 = BLK * b
                    L, G, R, W = LS[b], GS[b], RS[b], WS[b]
                    pss = pss_pool.tile([D, N], F32, tag="pss")
                    for rr in range((R + 1) // 2):
                        nrounds = min(2, R - 2 * rr)
                        psz = psz_pool.tile([D, 2 * N], F32, tag="psz")
                        h2 = h2pool.tile([D, 2 * N], BF16, tag="h2")
                        for rh in range(nrounds):
                            r = 2 * rr + rh
                            hs0 = hpool.tile([D, N], BF16, tag="hs0")
                            hs1 = hpool.tile([D, N], BF16, tag="hs1")
                            for g in range(G):
                                u = r * G + g  # pair index within block
                                for half in range(2):
                                    m = 2 * u + half  # local row 0..7
                                    hs = hs1 if half else hs0
                                    eng = (
                                        nc.gpsimd if m in GPS_ROWS else nc.vector
                                    )
                                    eng.tensor_scalar(
                                        out=hs[:, g * L : (g + 1) * L],
                                        in0=ct[:, j0:N],
                                        scalar1=abt[:, b * RPB + m : b * RPB + m + 1],
                                        scalar2=0.0,
                                        op0=ALU.add,
                                        op1=ALU.max,
                                    )
                            nc.tensor.matmul(
                                psz[:, rh * N : rh * N + W],
                                s_w2z0,
                                hs0[:, :W],
                                start=True,
                                stop=False,
                            )
                            nc.tensor.matmul(
                                psz[:, rh * N : rh * N + W],
                                s_w2z1,
                                hs1[:, :W],
                                start=False,
                                stop=True,
                            )
                        if nrounds == 2 and RELU2_ON_DVE(q, b):
                            nc.vector.tensor_scalar(
                                out=h2[:, : 2 * W].rearrange(
                                    "p (s w) -> p s w", s=2
                                ),
                                in0=psz[:, :].rearrange("p (s w) -> p s w", s=2)[
                                    :, :, :W
                                ],
                                scalar1=s_b2s,
                                scalar2=0.0,
                                op0=ALU.add,
                                op1=ALU.max,
                            )
                        elif nrounds == 2:
                            nc.scalar.activation(
                                out=h2[:, : 2 * W].rearrange(
                                    "p (s w) -> p s w", s=2
                                ),
                                in_=psz[:, :].rearrange("p (s w) -> p s w", s=2)[
                                    :, :, :W
                                ],
                                func=AF.Relu,
                                bias=s_b2s,
                            )
                        elif RELU2_ON_DVE(q, b):
                            nc.vector.tensor_scalar(
                                out=h2[:, :W],
                                in0=psz[:, :W],
                                scalar1=s_b2s,
                                scalar2=0.0,
                                op0=ALU.add,
                                op1=ALU.max,
                            )
                        else:
                            nc.scalar.activation(
                                out=h2[:, :W],
                                in_=psz[:, :W],
                                func=AF.Relu,
                                bias=s_b2s,
                            )
                        for rh in range(nrounds):
                            r = 2 * rr + rh
                            nc.tensor.matmul(
                                pss[:, :W],
                                s_wb[:, (2 + r) * D : (3 + r) * D],
                                h2[:, rh * W : rh * W + W],
                                start=(r == 0),
                                stop=(r == R - 1),
                            )
                    cb = SIGBASE[b]
                    nc.scalar.activation(
                        out=sig[:, cb : cb + W],
                        in_=pss[:, :W],
                        func=AF.Sigmoid,
                        bias=s_b3,
                    )
                    # ship everything but block 7 while block 7 computes
                    if b == NB - 2:
                        for a in range(4):
                            nc.sync.dma_start(
                                out=out[q, a, :, : SHIP1[a]],
                                in_=sig[32 * a : 32 * a + 2, : SHIP1[a]],
                            )
                # block-7 scores live in partitions 0:2 only (R=1)
                nc.sync.dma_start(
                    out=out[q, 0, :, SIGBASE[7] :],
                    in_=sig[0:2, SIGBASE[7] :],
                )

    nc.compile()
    return nc


def build_in_maps(dom, evo, W1, b1, W2, b2, W3, b3):
    # wb = [W2|0] [0|W2] w3q0..w3q3
    w2z0 = np.zeros((D, D), np.float32)
    w2z0[:, : D // 2] = W2
    w2z1 = np.zeros((D, D), np.float32)
    w2z1[:, D // 2 :] = W2
    w3qs = []
    for r in range(4):
        w3q = np.zeros((D, D), np.float32)
        w3q[: D // 2, 32 * r] = W3[:, 0]
        w3q[D // 2 :, 32 * r + 1] = W3[:, 0]
        w3qs.append(w3q)
    wb = np.concatenate([w2z0, w2z1] + w3qs, axis=1).astype(bf16_np)
    wf = np.ascontiguousarray(np.concatenate([W1[:D], W1[D:]], axis=1)).astype(bf16_np)
    bbt = np.zeros((D, 3), np.float32)
    bbt[:, 0] = b1
    bbt[:, 1] = np.concatenate([b2, b2])
    bbt[:, 2] = float(b3[0])

    in_maps = []
    for k in range(8):
        rows = np.concatenate(
            [BLK * bb_ + RPB * k + np.arange(RPB) for bb_ in range(NB)]
        )
        dom_iT = np.concatenate([dom[q][rows].T for q in range(B)], axis=1)
        evo_iT = np.concatenate([evo[q][rows].T for q in range(B)], axis=1)
        din = np.ascontiguousarray(
            np.concatenate(
                [
                    dom[0].T,
                    evo[0].T,
                    dom_iT,
                    evo_iT,
                    dom[1].T,
                    evo[1].T,
                ],
                axis=1,
            )
        ).astype(bf16_np)
        in_maps.append({"din": din, "wf": wf, "bb": bbt, "wb": wb})
    return in_maps


def unpack_results(results):
    S = np.zeros((B, N, N), np.float32)
    for k in range(8):
        o = np.asarray(results[k]["out"], dtype=np.float32)  # [B, 4, 2, SIGW]
        for q in range(B):
            for b in range(NB):
                L, G, R, W = LS[b], GS[b], RS[b], WS[b]
                seg = o[q, :, :, SIGBASE[b] : SIGBASE[b] + W]  # [4, 2, W]
                for r in range(R):
                    for g in range(G):
                        u = r * G + g
                        i = BLK * b + RPB * k + 2 * u
                        S[q, i, BLK * b : BLK * b + L] = seg[r, 0, g * L : (g + 1) * L]
                        S[q, i + 1, BLK * b : BLK * b + L] = seg[
                            r, 1, g * L : (g + 1) * L
                        ]
    upper = np.triu(S, 1)
    return (upper + upper.transpose(0, 2, 1)).astype(np.float32)


def kernel(
    domain_features,
    evolutionary_features,
    W1,
    b1,
    W2,
    b2,
    W3,
    b3,
):
    global LAST_RESULT
    dom = np.ascontiguousarray(np.asarray(domain_features, dtype=np.float32))
    evo = np.ascontiguousarray(np.asarray(evolutionary_features, dtype=np.float32))
    W1 = np.asarray(W1, dtype=np.float32)
    b1 = np.asarray(b1, dtype=np.float32)
    W2 = np.asarray(W2, dtype=np.float32)
    b2 = np.asarray(b2, dtype=np.float32)
    W3 = np.asarray(W3, dtype=np.float32)
    b3 = np.asarray(b3, dtype=np.float32)

    nc = _build()
    in_maps = build_in_maps(dom, evo, W1, b1, W2, b2, W3, b3)

    trace = os.environ.get("KERNEL_TRACE", "0") == "1"
    res = run_bass_kernel_spmd(nc, in_maps, core_ids=list(range(8)), trace=trace)
    LAST_RESULT = res

    return unpack_results(res.results)


# revision 5
# speedup vs baseline: 3.1432x; 3.1432x over previous
"""Trainium2 Bass kernel for nn_CoevolutionAnalyzer (pairwise-MLP coevolution scores).

Math (per batch q):
    g = domain * evo                         [512, 128]
    a = g @ W1[:128], c = g @ W1[128:]       [512, 128]
    h_ij  = relu(a_i + c_j + b1)             [128]
    z2    = W2.T h_ij + b2 ; h2 = relu(z2)   [64]
    s_ij  = sigmoid(W3.h2 + b3)
    out   = triu(s,1) + triu(s,1).T

Sharding (8 cores, one SPMD program):
    Only j >= 64*floor(i/64) is computed (upper triangle padded to the 64-column
    block grid; the pad is discarded on the host via triu). Every core takes 8
    rows of every 64-row block (rows 64*b + 8*k + m) for both batches, so all
    cores run the identical instruction stream; only DMA'd data differs.

Per block (8 rows = 4 pairs, j-window [64b, 512) of length L):
    stage1 (DVE/GpSimd): h = relu(cT + a_i + b1) as bf16, two stacked tiles
            (even rows / odd rows), G pairs side by side (G*L <= 512)
    stage2 (PE bf16):   two accumulating 128-wide matmuls with stationaries
            [W2|0] and [0|W2] -> psz[0:64]=z2_even, psz[64:128]=z2_odd.
            128-wide stationary => Fast Weight Load => LDWEIGHTS hidden.
    relu2  (ACT or DVE): h2 = relu(psum + [b2;b2]) -> bf16
    stage3 (PE bf16):   per round r one 128-wide matmul with stationary w3q_r
            (W3-even at col 32r, W3-odd at col 32r+1, zeros elsewhere),
            accumulated into pss over the R rounds of the block.
    sigmoid(ACT):       bf16 into a per-batch staging tile [128, 3328]
    out:                partitions 32r+{0,1} shipped per batch, trimmed per r
"""

import os

import numpy as np
from ml_dtypes import bfloat16 as bf16_np

import concourse.bass as bass
import concourse.tile as tile
from concourse import bacc, mybir
from concourse.bass_utils import run_bass_kernel_spmd

B = 2
N = 512
D = 128
NB = 8          # number of 64-row j-blocks
BLK = N // NB   # 64
RPB = 8         # rows per core per block
NI = NB * RPB   # i-rows per core per batch (64)
F32 = mybir.dt.float32
BF16 = mybir.dt.bfloat16
AF = mybir.ActivationFunctionType
ALU = mybir.AluOpType

# per-block geometry
LS = [N - BLK * b for b in range(NB)]            # j-window lengths
GS = [min(4, N // L) for L in LS]                # pairs per round
RS = [4 // g for g in GS]                        # rounds per block
WS = [GS[b] * LS[b] for b in range(NB)]          # sig segment widths
SIGW = sum(WS)                                   # 3328
SIGBASE = [sum(WS[:b]) for b in range(NB)]

# ---- engine balance knobs ----
GPS_ROWS = ()   # GpSimd tensor_scalar measured ~11ns/col and starves DVE's
                # shared SBUF ports -- never route stage-1 there


def RELU2_ON_DVE(q, b):
    # ACT's activation tables finish loading ~9.5us in; route the first
    # blocks' relu2 to DVE so the pipeline isn't stalled on ACT
    return q == 0 and b in (0, 1)


# din column layout (host-reordered for early critical path)
DJ0, EJ0, DI, EI, DJ1, EJ1 = 0, N, 2 * N, 2 * N + NI * B, 2 * N + 2 * NI * B, 3 * N + 2 * NI * B
DIN_COLS = 4 * N + 2 * NI * B  # 2304

# how much of each a-strip is valid after block 6 (r < R_b blocks only)
SHIP1 = [SIGBASE[7], SIGBASE[6], SIGBASE[4], SIGBASE[4]]

LAST_RESULT = None  # set by kernel(); test harness reads exec_time_ns


def _build():
    nc = bacc.Bacc("TRN2", target_bir_lowering=False, debug=False, num_devices=8)

    din = nc.declare_dram_parameter("din", [D, DIN_COLS], BF16, isOutput=False)
    wf = nc.declare_dram_parameter("wf", [D, 2 * D], BF16, isOutput=False)    # w1a|w1b
    bb = nc.declare_dram_parameter("bb", [D, 3], F32, isOutput=False)         # b1|b2s|b3
    # wb: w2z0|w2z1|w3q0|w3q1|w3q2|w3q3  (each 128 cols)
    wb = nc.declare_dram_parameter("wb", [D, 6 * D], BF16, isOutput=False)
    out = nc.declare_dram_parameter("out", [B, 4, 2, SIGW], BF16, isOutput=True)

    with tile.TileContext(nc) as tc:
        with (
            tc.tile_pool(name="singles", bufs=1) as singles,
            tc.tile_pool(name="per_batch", bufs=2) as per_batch,
            tc.tile_pool(name="hpool", bufs=6) as hpool,
            tc.tile_pool(name="h2pool", bufs=3) as h2pool,
            tc.tile_pool(name="psz", bufs=2, space="PSUM") as psz_pool,
            tc.tile_pool(name="pss", bufs=2, space="PSUM") as pss_pool,
            tc.tile_pool(name="pset", bufs=1, space="PSUM") as pset_pool,
        ):
            s_in = singles.tile([D, DIN_COLS], BF16)
            s_wf = singles.tile([D, 2 * D], BF16)
            s_bb = singles.tile([D, 3], F32)
            s_wb = singles.tile([D, 6 * D], BF16)
            # critical-path input DMAs: batch-0 j-side first (SP queue),
            # weights/biases on ACT queue behind the table load
            nc.sync.dma_start(out=s_in[:, 0 : 2 * N], in_=din[:, 0 : 2 * N])
            nc.sync.dma_start(out=s_wf, in_=wf[:])
            nc.sync.dma_start(
                out=s_in[:, DI : DI + 2 * NI * B], in_=din[:, DI : DI + 2 * NI * B]
            )
            nc.scalar.dma_start(out=s_bb, in_=bb[:])
            nc.scalar.dma_start(out=s_wb, in_=wb[:])
            nc.scalar.dma_start(out=s_in[:, DJ1:], in_=din[:, DJ1:])

            s_w2z0 = s_wb[:, 0:D]
            s_w2z1 = s_wb[:, D : 2 * D]
            s_b1 = s_bb[:, 0:1]
            s_b2s = s_bb[:, 1:2]
            s_b3 = s_bb[:, 2:3]

            for q in range(B):
                # --- per-batch setup: gT, aT(+b1), cT ---
                gti = per_batch.tile([D, NI], BF16, tag="gti")
                nc.vector.tensor_mul(
                    gti,
                    s_in[:, DI + q * NI : DI + (q + 1) * NI],
                    s_in[:, EI + q * NI : EI + (q + 1) * NI],
                )
                dj = DJ0 if q == 0 else DJ1
                ej = EJ0 if q == 0 else EJ1
                gtj = per_batch.tile([D, N], BF16, tag="gtj")
                nc.vector.tensor_mul(
                    gtj, s_in[:, dj : dj + N], s_in[:, ej : ej + N]
                )
                ps_c = pset_pool.tile([D, N], F32, tag="ps_c")
                nc.tensor.matmul(ps_c[:], s_wf[:, D:], gtj[:])
                ps_a = pset_pool.tile([D, NI], F32, tag="ps_a")
                nc.tensor.matmul(ps_a[:], s_wf[:, :D], gti[:])
                ct = per_batch.tile([D, N], BF16, tag="ct")
                nc.scalar.copy(ct, ps_c[:])
                abt = per_batch.tile([D, NI], F32, tag="abt")
                nc.vector.tensor_scalar_add(abt, ps_a[:], s_b1)

                sig = per_batch.tile([D, SIGW], BF16, tag="sig")

                # --- j-block loop ---
                for b in range(NB):
                    j0 = BLK * b
                    L, G, R, W = LS[b], GS[b], RS[b], WS[b]
                    pss = pss_pool.tile([D, N], F32, tag="pss")
                    for rr in range((R + 1) // 2):
                        nrounds = min(2, R - 2 * rr)
                        psz = psz_pool.tile([D, 2 * N], F32, tag="psz")
                        h2 = h2pool.tile([D, 2 * N], BF16, tag="h2")
                        for rh in range(nrounds):
                            r = 2 * rr + rh
                            hs0 = hpool.tile([D, N], BF16, tag="hs0")
                            hs1 = hpool.tile([D, N], BF16, tag="hs1")
                            for g in range(G):
                                u = r * G + g  # pair index within block
                                for half in range(2):
                                    m = 2 * u + half  # local row 0..7
                                    hs = hs1 if half else hs0
                                    eng = (
                                        nc.gpsimd if m in GPS_ROWS else nc.vector
                                    )
                                    eng.tensor_scalar(
                                        out=hs[:, g * L : (g + 1) * L],
                                        in0=ct[:, j0:N],
                                        scalar1=abt[:, b * RPB + m : b * RPB + m + 1],
                                        scalar2=0.0,
                                        op0=ALU.add,
                                        op1=ALU.max,
                                    )
                            nc.tensor.matmul(
                                psz[:, rh * N : rh * N + W],
                                s_w2z0,
                                hs0[:, :W],
                                start=True,
                                stop=False,
                            )
                            nc.tensor.matmul(
                                psz[:, rh * N : rh * N + W],
                                s_w2z1,
                                hs1[:, :W],
                                start=False,
                                stop=True,
                            )
                        if nrounds == 2 and RELU2_ON_DVE(q, b):
                            nc.vector.tensor_scalar(
                                out=h2[:, : 2 * W].rearrange(
                                    "p (s w) -> p s w", s=2
                                ),
                                in0=psz[:, :].rearrange("p (s w) -> p s w", s=2)[
                                    :, :, :W
                                ],
                                scalar1=s_b2s,
                                scalar2=0.0,
                                op0=ALU.add,
                                op1=ALU.max,
                            )
                        elif nrounds == 2:
                            nc.scalar.activation(
                                out=h2[:, : 2 * W].rearrange(
                                    "p (s w) -> p s w", s=2
                                ),
                                in_=psz[:, :].rearrange("p (s w) -> p s w", s=2)[
                                    :, :, :W
                                ],
                                func=AF.Relu,
                                bias=s_b2s,
                            )
                        elif RELU2_ON_DVE(q, b):
                            nc.vector.tensor_scalar(
                                out=h2[:, :W],
                                in0=psz[:, :W],
                                scalar1=s_b2s,
                                scalar2=0.0,
                                op0=ALU.add,
                                op1=ALU.max,
                            )
                        else:
                            nc.scalar.activation(
                                out=h2[:, :W],
                                in_=psz[:, :W],
                                func=AF.Relu,
                                bias=s_b2s,
                            )
                        for rh in range(nrounds):
                            r = 2 * rr + rh
                            nc.tensor.matmul(
                                pss[:, :W],
                                s_wb[:, (2 + r) * D : (3 + r) * D],
                                h2[:, rh * W : rh * W + W],
                                start=(r == 0),
                                stop=(r == R - 1),
                            )
                    cb = SIGBASE[b]
                    nc.scalar.activation(
                        out=sig[:, cb : cb + W],
                        in_=pss[:, :W],
                        func=AF.Sigmoid,
                        bias=s_b3,
                    )
                    # ship everything but block 7 while block 7 computes
                    if b == NB - 2:
                        for a in range(4):
                            nc.sync.dma_start(
                                out=out[q, a, :, : SHIP1[a]],
                                in_=sig[32 * a : 32 * a + 2, : SHIP1[a]],
                            )
                # block-7 scores live in partitions 0:2 only (R=1)
                nc.sync.dma_start(
                    out=out[q, 0, :, SIGBASE[7] :],
                    in_=sig[0:2, SIGBASE[7] :],
                )

    nc.compile()
    return nc


def build_in_maps(dom, evo, W1, b1, W2, b2, W3, b3):
    # wb = [W2|0] [0|W2] w3q0..w3q3
    w2z0 = np.zeros((D, D), np.float32)
    w2z0[:, : D // 2] = W2
    w2z1 = np.zeros((D, D), np.float32)
    w2z1[:, D // 2 :] = W2
    w3qs = []
    for r in range(4):
        w3q = np.zeros((D, D), np.float32)
        w3q[: D // 2, 32 * r] = W3[:, 0]
        w3q[D // 2 :, 32 * r + 1] = W3[:, 0]
        w3qs.append(w3q)
    wb = np.concatenate([w2z0, w2z1] + w3qs, axis=1).astype(bf16_np)
    wf = np.ascontiguousarray(np.concatenate([W1[:D], W1[D:]], axis=1)).astype(bf16_np)
    bbt = np.zeros((D, 3), np.float32)
    bbt[:, 0] = b1
    bbt[:, 1] = np.concatenate([b2, b2])
    bbt[:, 2] = float(b3[0])

    in_maps = []
    for k in range(8):
        rows = np.concatenate(
            [BLK * bb_ + RPB * k + np.arange(RPB) for bb_ in range(NB)]
        )
        dom_iT = np.concatenate([dom[q][rows].T for q in range(B)], axis=1)
        evo_iT = np.concatenate([evo[q][rows].T for q in range(B)], axis=1)
        din = np.ascontiguousarray(
            np.concatenate(
                [
                    dom[0].T,
                    evo[0].T,
                    dom_iT,
                    evo_iT,
                    dom[1].T,
                    evo[1].T,
                ],
                axis=1,
            )
        ).astype(bf16_np)
        in_maps.append({"din": din, "wf": wf, "bb": bbt, "wb": wb})
    return in_maps


def unpack_results(results):
    S = np.zeros((B, N, N), np.float32)
    for k in range(8):
        o = np.asarray(results[k]["out"], dtype=np.float32)  # [B, 4, 2, SIGW]
        for q in range(B):
            for b in range(NB):
                L, G, R, W = LS[b], GS[b], RS[b], WS[b]
                seg = o[q, :, :, SIGBASE[b] : SIGBASE[b] + W]  # [4, 2, W]
                for r in range(R):
                    for g in range(G):
                        u = r * G + g
                        i = BLK * b + RPB * k + 2 * u
                        S[q, i, BLK * b : BLK * b + L] = seg[r, 0, g * L : (g + 1) * L]
                        S[q, i + 1, BLK * b : BLK * b + L] = seg[
                            r, 1, g * L : (g + 1) * L
                        ]
    upper = np.triu(S, 1)
    return (upper + upper.transpose(0, 2, 1)).astype(np.float32)


def kernel(
    domain_features,
    evolutionary_features,
    W1,
    b1,
    W2,
    b2,
    W3,
    b3,
):
    global LAST_RESULT
    dom = np.ascontiguousarray(np.asarray(domain_features, dtype=np.float32))
    evo = np.ascontiguousarray(np.asarray(evolutionary_features, dtype=np.float32))
    W1 = np.asarray(W1, dtype=np.float32)
    b1 = np.asarray(b1, dtype=np.float32)
    W2 = np.asarray(W2, dtype=np.float32)
    b2 = np.asarray(b2, dtype=np.float32)
    W3 = np.asarray(W3, dtype=np.float32)
    b3 = np.asarray(b3, dtype=np.float32)

    nc = _build()
    in_maps = build_in_maps(dom, evo, W1, b1, W2, b2, W3, b3)

    trace = os.environ.get("KERNEL_TRACE", "0") == "1"
    res = run_bass_kernel_spmd(nc, in_maps, core_ids=list(range(8)), trace=trace)
    LAST_RESULT = res

    return unpack_results(res.results)
